# revision 3
# baseline (speedup 1.0000x reference)
"""Trainium2 distributed kernel for windowed (local-p) attention.

Module (S=4096 src positions, B=128 batch, H=128 dim):
    scores[s,b] = <e[s,b,:], (d @ W_a)[b,:]>          # full pass over e (268 MB)
    a = softmax(scores, axis=s)
    p_t[b] = S * sigmoid(tanh(d @ W_p) @ v_p)         # predicted center
    w = a * exp(-(p_t-s)^2/2) * [|p_t-s| <= 2]        # 5-wide window
    context[b] = sum_s w[s,b] * e[s,b,:]              # touches <=5 rows per b
    output = tanh([context, d] @ W_c)

Memory-bound: the roofline is one full read of e. Sharding: data-parallel
over batch, 16 batches per core, no cross-core communication.

Device kernel (per core):
  - host pre-transposes the e shard to [chunk=8][b=16][h=128][s=512] so each
    (chunk, b) block is a contiguous 256 KB DMA and h lands on partitions.
  - scores via TensorE: stationary = per-batch masked q columns  [128h, 16b]
    (LDWEIGHTS of 16 cols is ~free), moving = e block [128h, 512s].  The 16
    per-batch matmuls accumulate into one PSUM tile [16, 512]; the mask
    zeroes every output row except the matmul's own batch row, so the
    accumulated tile is the exact scores^T chunk with no extraction step.
  - softmax: per-chunk running max on DVE during the stream; tail does
    exp(scores - max) on ScalarE with accum_out giving Z for free, then one
    fused DVE op (ew * 1/Z) * gauss_mask -> w^T, DMA out.
Host computes the tiny parts: q/p_t/gauss-window (inputs massage), and the
context gather + output projection (<=5 e-rows per batch) from the returned w.
"""

import os
import sys

import numpy as np

sys.path.insert(0, "/opt/trn_rl_repo")

S, B, H = 4096, 128, 128
NCORES = 8
BPC = B // NCORES          # batches per core = 16
NCHUNK = 8                 # s-chunks
SW = S // NCHUNK           # 512 positions per chunk
D_WIN = 2.0
SIGMA = D_WIN / 2.0

_COMPILED = None           # (nc, ) cache so repeat calls skip compile
LAST_RESULT = None         # BassKernelResults of the last run (for test.py)


def _build_program():
    import concourse.tile as tile
    from concourse import bacc, mybir

    f32 = mybir.dt.float32
    nc = bacc.Bacc("TRN2", target_bir_lowering=False, debug=False,
                   num_devices=NCORES)

    et = nc.dram_tensor("et", [NCHUNK, BPC, H, SW], f32, kind="ExternalInput").ap()
    qm = nc.dram_tensor("qm", [H, BPC * BPC], f32, kind="ExternalInput").ap()
    gm = nc.dram_tensor("gm", [BPC, S], f32, kind="ExternalInput").ap()
    wt = nc.dram_tensor("wt", [BPC, S], f32, kind="ExternalOutput").ap()

    with tile.TileContext(nc) as tc:
        with (
            tc.tile_pool(name="eb", bufs=2) as epool,
            tc.tile_pool(name="keep", bufs=1) as keep,
            tc.tile_pool(name="ps", bufs=2, space="PSUM") as pspool,
        ):
            qm_t = keep.tile([H, BPC * BPC], f32, tag="qm")
            nc.sync.dma_start(qm_t[:], qm)
            gm_t = keep.tile([BPC, S], f32, tag="gm")
            nc.sync.dma_start(gm_t[:], gm)

            scoresT = keep.tile([BPC, S], f32, tag="scoresT")
            cmax = keep.tile([BPC, NCHUNK], f32, tag="cmax")

            for c in range(NCHUNK):
                etile = epool.tile([H, BPC, SW], f32, tag="et")
                for b in range(BPC):
                    # contiguous 256 KB block per (chunk, batch)
                    nc.sync.dma_start(etile[:, b, :], et[c, b])
                ps = pspool.tile([BPC, SW], f32, tag="ps")
                for b in range(BPC):
                    nc.tensor.matmul(
                        ps[:],
                        qm_t[:, b * BPC:(b + 1) * BPC],
                        etile[:, b, :],
                        start=(b == 0),
                        stop=(b == BPC - 1),
                    )
                nc.scalar.copy(scoresT[:, c * SW:(c + 1) * SW], ps[:])
                nc.vector.reduce_max(cmax[:, c:c + 1], ps[:],
                                     axis=mybir.AxisListType.X)

            gmax = keep.tile([BPC, 1], f32, tag="gmax")
            nc.vector.reduce_max(gmax[:], cmax[:], axis=mybir.AxisListType.X)
            negmax = keep.tile([BPC, 1], f32, tag="negmax")
            nc.vector.tensor_scalar_mul(negmax[:], gmax[:], -1.0)

            ew = keep.tile([BPC, S], f32, tag="ew")
            zsum = keep.tile([BPC, 1], f32, tag="zsum")
            nc.scalar.activation(ew[:], scoresT[:],
                                 mybir.ActivationFunctionType.Exp,
                                 bias=negmax[:], scale=1.0, accum_out=zsum[:])
            rz = keep.tile([BPC, 1], f32, tag="rz")
            nc.vector.reciprocal(rz[:], zsum[:])

            wt_t = keep.tile([BPC, S], f32, tag="wt")
            # w^T = (ew * (1/Z)) * gauss_mask   -- one fused DVE op
            nc.vector.scalar_tensor_tensor(wt_t[:], ew[:], rz[:], gm_t[:],
                                           op0=mybir.AluOpType.mult,
                                           op1=mybir.AluOpType.mult)
            nc.sync.dma_start(wt, wt_t[:])

    nc.compile()
    return nc


def _get_program():
    global _COMPILED
    if _COMPILED is None:
        _COMPILED = _build_program()
    return _COMPILED


def _install_ntff_hook():
    """This image's `antenv` lacks `axon_hooks`, so trace=True degrades.
    Recreate the module and register the ctypes-based NTFF hook that
    trn_boot would have installed. Test-only path (BASS_KERNEL_TRACE=1)."""
    import types

    try:
        from antenv.axon_hooks import get_axon_ntff_profile_hook  # noqa: F401
        return
    except ImportError:
        pass
    import antenv
    from trn_agent_boot.trn_boot import _ntff_profile_via_ctypes

    mod = types.ModuleType("antenv.axon_hooks")
    mod._hook = _ntff_profile_via_ctypes("/opt/axon/libaxon_pjrt.so")
    mod.get_axon_ntff_profile_hook = lambda: mod._hook
    mod.set_axon_ntff_profile_hook = lambda h: setattr(mod, "_hook", h)
    sys.modules["antenv.axon_hooks"] = mod
    antenv.axon_hooks = mod

    # upload_artifacts needs bucket egress this container doesn't have.
    import concourse.bass_utils as bu
    orig_upload = bu.upload_artifacts

    def _safe_upload(tmpdir):
        try:
            return orig_upload(tmpdir)
        except Exception:
            return str(tmpdir)

    bu.upload_artifacts = _safe_upload


def kernel(e, d, W_a, W_p, v_p, W_c):
    global LAST_RESULT
    from concourse.bass_utils import run_bass_kernel_spmd

    e = np.asarray(e, dtype=np.float32)
    d = np.asarray(d, dtype=np.float32)
    W_a = np.asarray(W_a, dtype=np.float32)
    W_p = np.asarray(W_p, dtype=np.float32)
    v_p = np.asarray(v_p, dtype=np.float32)
    W_c = np.asarray(W_c, dtype=np.float32)

    d0 = d[0]                                   # [B, H]
    q = d0 @ W_a                                # [B, H]
    p_t = (S * _sigmoid(np.tanh(d0 @ W_p) @ v_p)).reshape(B)   # [B]

    pos = np.arange(S, dtype=np.float32)        # [S]
    diff = p_t[:, None] - pos[None, :]          # [B, S]
    mask = (np.abs(diff) <= D_WIN)
    gaussT = (np.exp(-(diff.astype(np.float32) ** 2) / np.float32(2.0 * SIGMA ** 2))
              * mask).astype(np.float32)        # [B, S]

    in_maps = []
    for i in range(NCORES):
        bs = slice(i * BPC, (i + 1) * BPC)
        # e[:, bs, :] -> [chunk, b, h, s_local]
        esh = e[:, bs, :].reshape(NCHUNK, SW, BPC, H).transpose(0, 2, 3, 1)
        qmask = np.zeros((H, BPC * BPC), dtype=np.float32)
        for b in range(BPC):
            qmask[:, b * BPC + b] = q[i * BPC + b]
        in_maps.append({
            "et": np.ascontiguousarray(esh),
            "qm": qmask,
            "gm": np.ascontiguousarray(gaussT[bs]),
        })

    nc = _get_program()
    trace = bool(int(os.environ.get("BASS_KERNEL_TRACE", "0")))
    if trace:
        _install_ntff_hook()
    res = run_bass_kernel_spmd(nc, in_maps, core_ids=list(range(NCORES)),
                               trace=trace)
    LAST_RESULT = res

    w = np.zeros((S, B), dtype=np.float32)
    for i in range(NCORES):
        bs = slice(i * BPC, (i + 1) * BPC)
        w[:, bs] = res.results[i]["wt"].T

    context = np.zeros((B, H), dtype=np.float32)
    for b in range(B):
        rows = np.nonzero(mask[b])[0]
        context[b] = w[rows, b].astype(np.float32) @ e[rows, b, :]

    x = np.concatenate([context[None], d], axis=2)       # [1, B, 2H]
    output = np.tanh(x @ W_c).astype(np.float32)         # [1, B, H]
    return output, w


def _sigmoid(x):
    return 1.0 / (1.0 + np.exp(-x.astype(np.float32), dtype=np.float32))


# revision 10
# speedup vs baseline: 1.0820x; 1.0820x over previous
"""Trainium2 distributed kernel for windowed (local-p) attention.

Module (S=4096 src positions, B=128 batch, H=128 dim):
    scores[s,b] = <e[s,b,:], (d @ W_a)[b,:]>          # full pass over e (268 MB)
    a = softmax(scores, axis=s)
    p_t[b] = S * sigmoid(tanh(d @ W_p) @ v_p)         # predicted center
    w = a * exp(-(p_t-s)^2/2) * [|p_t-s| <= 2]        # 5-wide window
    context[b] = sum_s w[s,b] * e[s,b,:]              # touches <=5 rows per b
    output = tanh([context, d] @ W_c)

Memory-bound: the roofline is one full read of e. Sharding: data-parallel
over batch, 16 batches per core, no cross-core communication.

Device kernel (per core):
  - host pre-transposes the e shard to [chunk=8][b=16][h=128][s=512] so each
    (chunk, b) block is a contiguous 256 KB DMA and h lands on partitions.
  - scores via TensorE: stationary = per-batch masked q columns  [128h, 16b]
    (LDWEIGHTS of 16 cols is ~free), moving = e block [128h, 512s].  The 16
    per-batch matmuls accumulate into one PSUM tile [16, 512]; the mask
    zeroes every output row except the matmul's own batch row, so the
    accumulated tile is the exact scores^T chunk with no extraction step.
  - softmax: per-chunk running max on DVE during the stream; tail does
    exp(scores - max) on ScalarE with accum_out giving Z for free, then one
    fused DVE op (ew * 1/Z) * gauss_mask -> w^T, DMA out.
Host computes the tiny parts: q/p_t/gauss-window (inputs massage), and the
context gather + output projection (<=5 e-rows per batch) from the returned w.
"""

import os
import sys

import numpy as np

sys.path.insert(0, "/opt/trn_rl_repo")

S, B, H = 4096, 128, 128
NCORES = 8
BPC = B // NCORES          # batches per core = 16
NCHUNK = 8                 # s-chunks
SW = S // NCHUNK           # 512 positions per chunk
D_WIN = 2.0
SIGMA = D_WIN / 2.0

_COMPILED = None           # (nc, ) cache so repeat calls skip compile
LAST_RESULT = None         # BassKernelResults of the last run (for test.py)


def _build_program():
    import concourse.tile as tile
    from concourse import bacc, mybir

    f32 = mybir.dt.float32
    bf16 = mybir.dt.bfloat16
    nc = bacc.Bacc("TRN2", target_bir_lowering=False, debug=False,
                   num_devices=NCORES)

    # e split into bf16 hi/lo halves: [chunk][hi/lo][b][h][s]
    et = nc.dram_tensor("et", [NCHUNK, 2, BPC, H, SW], bf16,
                        kind="ExternalInput").ap()
    # stationary q columns, bf16: per b a 48-col block
    #   col b*48 + b      = q_hi[b]   (pass-1 rows 0:16, also pass 2)
    #   col b*48 + 32 + b = q_lo[b]   (pass-1 rows 32:48 -- 32-aligned for
    #                                  the ScalarE PSUM read; rows 16:32 pad)
    qm = nc.dram_tensor("qm", [H, BPC * 48], bf16, kind="ExternalInput").ap()
    gm = nc.dram_tensor("gm", [BPC, S], f32, kind="ExternalInput").ap()
    wt = nc.dram_tensor("wt", [BPC, S], f32, kind="ExternalOutput").ap()

    with tile.TileContext(nc) as tc:
        with (
            tc.tile_pool(name="eb", bufs=3) as epool,
            tc.tile_pool(name="keep", bufs=1) as keep,
            tc.tile_pool(name="tmp", bufs=2) as tmp,
            tc.tile_pool(name="ps", bufs=2, space="PSUM") as pspool,
        ):
            qm_t = keep.tile([H, BPC * 48], bf16, tag="qm")
            nc.sync.dma_start(qm_t[:], qm)
            gm_t = keep.tile([BPC, S], f32, tag="gm")
            nc.sync.dma_start(gm_t[:], gm)

            scoresT = keep.tile([BPC, S], f32, tag="scoresT")
            cmax = keep.tile([BPC, NCHUNK], f32, tag="cmax")

            for c in range(NCHUNK):
                # one big DMA per half: spreads over all 16 SDMA engines;
                # hi on the sync HWDGE ring, lo on the scalar HWDGE ring.
                ehtile = epool.tile([H, BPC, SW], bf16, tag="eh")
                nc.sync.dma_start(ehtile[:], et[c, 0].rearrange("b h s -> h b s"))
                eltile = epool.tile([H, BPC, SW], bf16, tag="el")
                nc.sync.dma_start(eltile[:], et[c, 1].rearrange("b h s -> h b s"))

                psA = pspool.tile([48, SW], f32, tag="psA")
                for b in range(BPC):
                    # rows 0:16 += q_hi . e_hi ; rows 32:48 += q_lo . e_hi
                    nc.tensor.matmul(psA[:], qm_t[:, b * 48:(b + 1) * 48],
                                     ehtile[:, b, :], start=(b == 0),
                                     stop=(b == BPC - 1))
                psB = pspool.tile([BPC, SW], f32, tag="psB")
                for b in range(BPC):
                    # += q_hi . e_lo  (own accumulation group / bank)
                    nc.tensor.matmul(psB[:], qm_t[:, b * 48:b * 48 + BPC],
                                     eltile[:, b, :], start=(b == 0),
                                     stop=(b == BPC - 1))

                lo_t = tmp.tile([BPC, SW], f32, tag="lo")
                nc.scalar.copy(lo_t[:], psA[32:48, :])
                t1 = tmp.tile([BPC, SW], f32, tag="t1")
                nc.vector.tensor_add(t1[:], psA[0:BPC, :], lo_t[:])
                sc = scoresT[:, c * SW:(c + 1) * SW]
                nc.vector.tensor_add(sc, t1[:], psB[:])
                nc.vector.reduce_max(cmax[:, c:c + 1], sc,
                                     axis=mybir.AxisListType.X)

            gmax = keep.tile([BPC, 1], f32, tag="gmax")
            nc.vector.reduce_max(gmax[:], cmax[:], axis=mybir.AxisListType.X)
            negmax = keep.tile([BPC, 1], f32, tag="negmax")
            nc.vector.tensor_scalar_mul(negmax[:], gmax[:], -1.0)

            ew = keep.tile([BPC, S], f32, tag="ew")
            zsum = keep.tile([BPC, 1], f32, tag="zsum")
            nc.scalar.activation(ew[:], scoresT[:],
                                 mybir.ActivationFunctionType.Exp,
                                 bias=negmax[:], scale=1.0, accum_out=zsum[:])
            rz = keep.tile([BPC, 1], f32, tag="rz")
            nc.vector.reciprocal(rz[:], zsum[:])

            wt_t = keep.tile([BPC, S], f32, tag="wt")
            # w^T = (ew * (1/Z)) * gauss_mask   -- one fused DVE op
            nc.vector.scalar_tensor_tensor(wt_t[:], ew[:], rz[:], gm_t[:],
                                           op0=mybir.AluOpType.mult,
                                           op1=mybir.AluOpType.mult)
            nc.sync.dma_start(wt, wt_t[:])

    nc.compile()
    return nc


def _get_program():
    global _COMPILED
    if _COMPILED is None:
        _COMPILED = _build_program()
    return _COMPILED


def _install_ntff_hook():
    """This image's `antenv` lacks `axon_hooks`, so trace=True degrades.
    Recreate the module and register the ctypes-based NTFF hook that
    trn_boot would have installed. Test-only path (BASS_KERNEL_TRACE=1)."""
    import types

    try:
        from antenv.axon_hooks import get_axon_ntff_profile_hook  # noqa: F401
        return
    except ImportError:
        pass
    import antenv
    from trn_agent_boot.trn_boot import _ntff_profile_via_ctypes

    mod = types.ModuleType("antenv.axon_hooks")
    mod._hook = _ntff_profile_via_ctypes("/opt/axon/libaxon_pjrt.so")
    mod.get_axon_ntff_profile_hook = lambda: mod._hook
    mod.set_axon_ntff_profile_hook = lambda h: setattr(mod, "_hook", h)
    sys.modules["antenv.axon_hooks"] = mod
    antenv.axon_hooks = mod

    # upload_artifacts needs bucket egress this container doesn't have.
    import concourse.bass_utils as bu
    orig_upload = bu.upload_artifacts

    def _safe_upload(tmpdir):
        try:
            return orig_upload(tmpdir)
        except Exception:
            return str(tmpdir)

    bu.upload_artifacts = _safe_upload


def kernel(e, d, W_a, W_p, v_p, W_c):
    global LAST_RESULT
    from concourse.bass_utils import run_bass_kernel_spmd

    e = np.asarray(e, dtype=np.float32)
    d = np.asarray(d, dtype=np.float32)
    W_a = np.asarray(W_a, dtype=np.float32)
    W_p = np.asarray(W_p, dtype=np.float32)
    v_p = np.asarray(v_p, dtype=np.float32)
    W_c = np.asarray(W_c, dtype=np.float32)

    d0 = d[0]                                   # [B, H]
    q = d0 @ W_a                                # [B, H]
    p_t = (S * _sigmoid(np.tanh(d0 @ W_p) @ v_p)).reshape(B)   # [B]

    pos = np.arange(S, dtype=np.float32)        # [S]
    diff = p_t[:, None] - pos[None, :]          # [B, S]
    mask = (np.abs(diff) <= D_WIN)
    gaussT = (np.exp(-(diff.astype(np.float32) ** 2) / np.float32(2.0 * SIGMA ** 2))
              * mask).astype(np.float32)        # [B, S]

    import ml_dtypes
    bf16 = ml_dtypes.bfloat16
    q_hi = q.astype(bf16)
    q_lo = (q - q_hi.astype(np.float32)).astype(bf16)

    in_maps = []
    for i in range(NCORES):
        bs = slice(i * BPC, (i + 1) * BPC)
        # e[:, bs, :] -> [chunk, b, h, s_local], then bf16 hi/lo split
        esh = np.ascontiguousarray(
            e[:, bs, :].reshape(NCHUNK, SW, BPC, H).transpose(0, 2, 3, 1))
        e_hi = esh.astype(bf16)
        e_lo = (esh - e_hi.astype(np.float32)).astype(bf16)
        ehl = np.stack([e_hi, e_lo], axis=1)          # [chunk, 2, b, h, s]
        qmask = np.zeros((H, BPC * 48), dtype=bf16)
        for b in range(BPC):
            qmask[:, b * 48 + b] = q_hi[i * BPC + b]
            qmask[:, b * 48 + 32 + b] = q_lo[i * BPC + b]
        in_maps.append({
            "et": np.ascontiguousarray(ehl),
            "qm": qmask,
            "gm": np.ascontiguousarray(gaussT[bs]),
        })

    nc = _get_program()
    trace = bool(int(os.environ.get("BASS_KERNEL_TRACE", "0")))
    if trace:
        _install_ntff_hook()
    res = run_bass_kernel_spmd(nc, in_maps, core_ids=list(range(NCORES)),
                               trace=trace)
    LAST_RESULT = res

    w = np.zeros((S, B), dtype=np.float32)
    for i in range(NCORES):
        bs = slice(i * BPC, (i + 1) * BPC)
        w[:, bs] = res.results[i]["wt"].T

    context = np.zeros((B, H), dtype=np.float32)
    for b in range(B):
        rows = np.nonzero(mask[b])[0]
        context[b] = w[rows, b].astype(np.float32) @ e[rows, b, :]

    x = np.concatenate([context[None], d], axis=2)       # [1, B, 2H]
    output = np.tanh(x @ W_c).astype(np.float32)         # [1, B, H]
    return output, w


def _sigmoid(x):
    return 1.0 / (1.0 + np.exp(-x.astype(np.float32), dtype=np.float32))
